# revision 6
# baseline (speedup 1.0000x reference)
"""Trainium2 Bass kernel for multi-head attention (B=2, T=2048, C=1024, H=16, DH=64).

Sharding: tensor-parallel over heads. Each of the 8 cores computes 2 heads:
q/k/v projections for its heads, attention, and a partial output projection
(its 128-column slice of the concat-head dim against its 128-row slice of Wp).
The host sums the 8 fp16 partial outputs in fp32 and adds the bias.

Numerics/engine strategy (validated against a float64 oracle, rel err ~9e-3
vs the 2e-2 gate):
  - projections run as fp8e4 DoubleRow matmuls on a 3-term hi/lo split:
    x = x1+x2, W*64 = W1+W2, q ~ (x1W1) + (x2W1 + x1W2), with the two cross
    terms packed into the two DoubleRow planes of a single instruction.
    6 instrs/chunk-pair-equivalent vs 8 fp32r chunks -> 0.75x PE cycles, and
    x streams from HBM as two fp8 planes (half the fp32 bytes).
  - scores run as fp8e4 DoubleRow with q split exact (q1,q2 planes) and k
    single-quantized: s = k1^T(q1+q2) at 0.5 cycles/row (2x fp32r). Both
    heads live on the free dim of [64,2,2,span] tiles so every matmul uses
    tile position (0,0) (alternating positions with DoubleRow faults).
  - exp on ACT emits bf16 tiles, two key chunks per instruction where the
    key mask allows, scale = 0.125/64 folded with the fp8 evac scales.
  - attention@V is transposed: exp tile is the stationary operand (128-query
    slices), [v|1] bf16 is moving (65-wide) -> full 128-partition output
    utilization plus a free softmax denominator column; normalization is a
    per-partition reciprocal+mul on DVE (no gpsimd broadcast).
  - normalized heads are re-transposed (bf16, via identity) and multiplied
    against bf16 Wp; psum is evacuated to fp16 on the otherwise-idle gpsimd
    engine and DMA'd out as fp16.
"""

from contextlib import ExitStack

import numpy as np

B, T, C, H, DH = 2, 2048, 1024, 16, 64
NCORES = 8
HP = H // NCORES          # heads per core
M = HP * DH               # 128 = packed head dim per core
P = 128                   # partitions / contraction chunk
QT = 512                  # query/token tile (free dim)
NEG = -30000.0            # additive mask value (exp(NEG + anything small) == 0)
KC_N = C // P             # contraction chunks for projections
ESC = 0.125 / 64.0        # exp scale: 1/sqrt(dh) / (q*8 * k*8)


def _build(lens, t=T, c=C):
    """Build the per-core Bass module for batch lengths `lens` (tuple of B ints)."""
    import concourse.mybir as mybir
    import concourse.tile as tile
    from concourse import bacc
    from concourse.masks import make_identity

    f32 = mybir.dt.float32
    bf16 = mybir.dt.bfloat16
    f16 = mybir.dt.float16
    fp8 = mybir.dt.float8e4
    AF = mybir.ActivationFunctionType
    PM = mybir.MatmulPerfMode
    ALU = mybir.AluOpType

    qt = min(QT, t)
    nkv = [(l + P - 1) // P for l in lens]         # valid key chunks == token blocks
    partial = [l % P != 0 for l in lens]
    crop = [n * P for n in nkv]                    # token coverage per batch
    nq = [(cr + qt - 1) // qt for cr in crop]      # query tiles per batch
    nkv_max = max(nkv)

    def tiw(b, i):
        return min(crop[b] - i * qt, qt)           # multiples of 128

    nc = bacc.Bacc("TRN2", target_bir_lowering=False, debug=False,
                   num_devices=NCORES)

    x8_d = nc.dram_tensor("x8", [P, KC_N, 2, B * t], fp8, kind="ExternalInput").ap()
    w_hi_d = [nc.dram_tensor(f"w{n}hi", [P, KC_N, M], fp8, kind="ExternalInput").ap()
              for n in ("q", "k", "v")]
    w_x_d = [nc.dram_tensor(f"w{n}x", [P, KC_N, 2, M], fp8, kind="ExternalInput").ap()
             for n in ("q", "k", "v")]
    wp_d = nc.dram_tensor("wp", [M, c], bf16, kind="ExternalInput").ap()
    km_d = nc.dram_tensor("km", [P, B], f32, kind="ExternalInput").ap()
    out_d = nc.dram_tensor("out", [B * t, c], f16, kind="ExternalOutput").ap()

    with tile.TileContext(nc) as tc, ExitStack() as ctx:
        const = ctx.enter_context(tc.tile_pool(name="const", bufs=1))
        persist = ctx.enter_context(tc.tile_pool(name="persist", bufs=1))

        identb = const.tile([P, P], bf16)
        make_identity(nc, identb[:])
        kmask = const.tile([P, B], f32)
        wp_sb = const.tile([P, c], bf16)

        # q8/k8: [64 dims, plane, head, token] so both heads' score matmuls
        # sit at tile position (0,0); k planes both hold k1 (duplicated).
        q8 = persist.tile([DH, 2, HP, B * t], fp8, tag="q8")
        k8 = persist.tile([DH, 2, HP, B * t], fp8, tag="k8")
        vTb = persist.tile([P, B * t], bf16, tag="vTb")
        vaug = [persist.tile([P, B, nkv_max, DH + 1], bf16, tag=f"vaug{h}",
                             name=f"vaug{h}")
                for h in range(HP)]

        with tc.tile_pool(name="wpool", bufs=1) as wpool, \
             tc.tile_pool(name="xpool", bufs=3) as xpool, \
             tc.tile_pool(name="exps", bufs=16) as expp, \
             tc.tile_pool(name="aob", bufs=2) as aobp, \
             tc.tile_pool(name="aot", bufs=2) as aotp, \
             tc.tile_pool(name="stage", bufs=2) as stage, \
             tc.tile_pool(name="recp", bufs=4) as recp, \
             tc.tile_pool(name="work", bufs=2, space="PSUM") as workp, \
             tc.tile_pool(name="psc", bufs=1, space="PSUM") as pscp, \
             tc.tile_pool(name="pav", bufs=2, space="PSUM") as pavp:

            w_hi = []
            w_x = []
            for i, n in enumerate(("q", "k", "v")):
                whi = wpool.tile([P, KC_N, M], fp8, tag=f"w{n}hi", name=f"w{n}hi")
                wx = wpool.tile([P, KC_N, 2, M], fp8, tag=f"w{n}x", name=f"w{n}x")
                w_hi.append(whi)
                w_x.append(wx)
            for h in range(HP):
                nc.vector.memset(vaug[h][:], 1.0)

            # scores psum: [slot-pair, head, qt] = 4 banks; exp reads pairs
            sc = pscp.tile([P, 2, HP, qt], f32, tag="sc")

            # Warm-up: dependency-free matmuls release the PE clock gate,
            # a dummy Exp preloads the ACT table set
            warm = workp.tile([P, qt], f32, tag="work", name="warm")
            for i in range(17):
                nc.tensor.matmul(warm[:, 0:P], identb[:], identb[:],
                                 start=(i == 0), stop=(i == 16))
            dummy = const.tile([P, P], f32, name="dummy")
            nc.scalar.activation(dummy[:], identb[:], AF.Exp)

            def emit_proj_tile(b, n):
                tok0 = b * t + n * qt
                tw = tiw(b, n)
                xtile = xpool.tile([P, KC_N, 2, qt], fp8, tag="x", name="xtile")
                if b == 0 and n == 0:
                    # weight DMAs first, x tile split so matmuls start early
                    nc.sync.dma_start(w_hi[0][:], w_hi_d[0][:])
                    nc.sync.dma_start(w_x[0][:], w_x_d[0][:])
                    nc.sync.dma_start(
                        xtile[:, 0:2, :, 0:tw], x8_d[:, 0:2, :, tok0:tok0 + tw])
                    nc.sync.dma_start(w_hi[1][:], w_hi_d[1][:])
                    nc.sync.dma_start(w_x[1][:], w_x_d[1][:])
                    nc.sync.dma_start(w_hi[2][:], w_hi_d[2][:])
                    nc.sync.dma_start(w_x[2][:], w_x_d[2][:])
                    nc.sync.dma_start(
                        xtile[:, 2:KC_N, :, 0:tw], x8_d[:, 2:KC_N, :, tok0:tok0 + tw])
                    nc.sync.dma_start(kmask[:], km_d[:])
                    nc.sync.dma_start(wp_sb[:], wp_d[:])
                else:
                    nc.sync.dma_start(
                        xtile[:, :, :, 0:tw], x8_d[:, :, :, tok0:tok0 + tw])
                for i in range(3):
                    ps = workp.tile([P, qt], f32, tag="work", name="ps")
                    # main term: x1@W1, two chunks per DoubleRow instr
                    for k in range(KC_N // 2):
                        nc.tensor.matmul(
                            ps[:, 0:tw],
                            w_hi[i][:, 2 * k:2 * k + 2, :],
                            xtile[:, 2 * k:2 * k + 2, 0, 0:tw],
                            start=(k == 0), stop=False, perf_mode=PM.DoubleRow)
                    # cross terms: planes (W2,x1),(W1,x2) per chunk
                    for k in range(KC_N):
                        nc.tensor.matmul(
                            ps[:, 0:tw],
                            w_x[i][:, k, :, :],
                            xtile[:, k, :, 0:tw],
                            start=False, stop=(k == KC_N - 1),
                            perf_mode=PM.DoubleRow)
                    span = slice(tok0, tok0 + tw)
                    if i == 0:      # q: plane0 = fp8(ps/8) on Pool, resid on DVE
                        for h in range(HP):
                            hsl = slice(h * DH, (h + 1) * DH)
                            nc.gpsimd.tensor_scalar(
                                q8[:, 0, h, span], ps[hsl, 0:tw], 0.125, None,
                                ALU.mult)
                            nc.vector.scalar_tensor_tensor(
                                q8[:, 1, h, span], ps[hsl, 0:tw], 0.125,
                                q8[:, 0, h, span], ALU.mult, ALU.subtract)
                    elif i == 1:    # k: single fp8, duplicated into both planes
                        for h in range(HP):
                            hsl = slice(h * DH, (h + 1) * DH)
                            nc.vector.tensor_scalar(
                                k8[:, 0, h, span], ps[hsl, 0:tw], 0.125, None,
                                ALU.mult)
                            nc.vector.tensor_copy(k8[:, 1, h, span],
                                                  k8[:, 0, h, span])
                    else:           # v: bf16 at true scale
                        nc.vector.tensor_scalar(
                            vTb[:, span], ps[:, 0:tw], 1.0 / 64.0, None,
                            ALU.mult)

            def emit_vaug_chunks(b, k0, k1):
                for k in range(k0, k1):
                    key0 = b * t + k * P
                    pt = workp.tile([P, qt], bf16, tag="work", name="pt")
                    nc.tensor.transpose(pt[:, 0:P], vTb[:, key0:key0 + P],
                                        identb[:])
                    for h in range(HP):
                        nc.vector.tensor_copy(vaug[h][:, b, k, 0:DH],
                                              pt[:, h * DH:(h + 1) * DH])

            def _exp_single(b, qw, k, etiles, bias):
                et = expp.tile([P, 2, HP, qt], bf16, tag="et", name="et")
                src = sc[:, k % 2:k % 2 + 1, :, 0:qw]
                dst = et[:, 0:1, :, 0:qw]
                if bias is None:
                    nc.scalar.activation(dst, src, AF.Exp, scale=ESC)
                else:
                    nc.scalar.activation(dst, src, AF.Exp, bias=bias,
                                         scale=ESC)
                etiles.append((k, 0, et))

            def emit_scores_chunks(b, q, k0, k1, etiles, pend):
                # pend: 1-slot list holding an un-exp'd even chunk index
                q0 = b * t + q * qt
                qw = tiw(b, q)
                for k in range(k0, k1):
                    key0 = b * t + k * P
                    slot = k % 2
                    for h in range(HP):
                        nc.tensor.matmul(
                            sc[:, slot, h, 0:qw],
                            k8[:, :, h, key0:key0 + P],
                            q8[:, :, h, q0:q0 + qw],
                            start=True, stop=True, perf_mode=PM.DoubleRow)
                    masked = partial[b] and k == nkv[b] - 1
                    if masked:
                        if pend[0] is not None:
                            _exp_single(b, qw, pend[0], etiles, None)
                            pend[0] = None
                        _exp_single(b, qw, k, etiles, kmask[:, b:b + 1])
                    elif pend[0] is not None:
                        # full unmasked pair (pend, k) in slots (0, 1)
                        et = expp.tile([P, 2, HP, qt], bf16, tag="et",
                                       name="et")
                        nc.scalar.activation(et[:, :, :, 0:qw],
                                             sc[:, :, :, 0:qw], AF.Exp,
                                             scale=ESC)
                        etiles.append((pend[0], 0, et))
                        etiles.append((k, 1, et))
                        pend[0] = None
                    elif k == nkv[b] - 1:
                        _exp_single(b, qw, k, etiles, None)
                    else:
                        pend[0] = k

            def emit_scores(b, q):
                etiles = []
                emit_scores_chunks(b, q, 0, nkv[b], etiles, [None])
                return etiles

            def emit_av_block(b, q, qb, etiles):
                # one 128-query block: AV for both heads, normalize,
                # re-transpose, output projection, fp16 DMA
                qw = tiw(b, q)
                col0 = qb * P
                aob = aobp.tile([P, P], bf16, tag="aob", name="aob")
                for h in range(HP):
                    pa = pavp.tile([P, qt], f32, tag="pa", name="pa")
                    nkb = nkv[b]
                    for idx, (k, cslot, et) in enumerate(etiles):
                        nc.tensor.matmul(
                            pa[:, 0:DH + 1],
                            et[:, cslot, h, col0:col0 + P],
                            vaug[h][:, b, k, :],
                            start=(idx == 0), stop=(idx == nkb - 1))
                    rec = recp.tile([P, 1], f32, tag="rec", name="rec")
                    nc.vector.reciprocal(rec[:], pa[:, DH:DH + 1])
                    nc.vector.tensor_scalar(
                        aob[:, h * DH:(h + 1) * DH], pa[:, 0:DH], rec[:],
                        None, ALU.mult)
                pat = workp.tile([P, qt], bf16, tag="work", name="pat")
                nc.tensor.transpose(pat[:, 0:P], aob[:], identb[:])
                aotT = aotp.tile([P, P], bf16, tag="aotT", name="aotT")
                nc.vector.tensor_copy(aotT[:], pat[:, 0:P])
                blk = q * (qt // P) + qb
                tok0 = b * t + blk * P
                final = (b == B - 1 and q == nq[b] - 1
                         and qb == qw // P - 1)
                st = stage.tile([P, c], f16, tag="st", name="st")
                qn = c // 2
                for nn in range(2):
                    po = workp.tile([P, qt], f32, tag="work", name="po")
                    nc.tensor.matmul(
                        po[:, 0:qn], aotT[:], wp_sb[:, nn * qn:(nn + 1) * qn],
                        start=True, stop=True)
                    nc.gpsimd.tensor_copy(st[:, nn * qn:(nn + 1) * qn],
                                          po[:, 0:qn])
                if final:
                    for j in range(4):
                        cs = slice(j * (c // 4), (j + 1) * (c // 4))
                        nc.sync.dma_start(out_d[tok0:tok0 + P, cs], st[:, cs])
                else:
                    nc.sync.dma_start(out_d[tok0:tok0 + P, :], st[:])

            # ---- emission schedule ----
            # b0 projections with early b0 score chunks + vaug transposes
            etiles0 = []
            pend0 = [None]
            k_sc = 0
            for n in range(nq[0]):
                emit_proj_tile(0, n)
                hi = min(((n + 1) * qt) // P, nkv[0])
                take = min(k_sc + 2, hi)
                emit_scores_chunks(0, 0, k_sc, take, etiles0, pend0)
                k_sc = take
            k_va = 0
            while k_sc < nkv[0] or k_va < nkv[0]:
                take = min(k_sc + 2, nkv[0])
                emit_scores_chunks(0, 0, k_sc, take, etiles0, pend0)
                k_sc = take
                take = min(k_va + 3, nkv[0])
                emit_vaug_chunks(0, k_va, take)
                k_va = take

            pend_av = []   # deferred (b, q, qb, etiles)
            ptiles = list(range(nq[1])) if B > 1 else []

            def drain_av(nmax):
                cnt = 0
                while pend_av and cnt < nmax:
                    emit_av_block(*pend_av.pop(0))
                    cnt += 1

            for b in range(B):
                for q in range(nq[b]):
                    ets = etiles0 if (b == 0 and q == 0) else emit_scores(b, q)
                    if b == 0 and ptiles:
                        emit_proj_tile(1, ptiles.pop(0))
                    nqb = tiw(b, q) // P
                    for qb in range(nqb):
                        pend_av.append((b, q, qb, ets))
                    # keep a backlog of ~3 blocks so the PE never starves
                    drain_av(max(0, len(pend_av) - 3))
                    if b == 0 and ptiles:
                        emit_proj_tile(1, ptiles.pop(0))
                    if b + 1 < B and q == nq[b] - 1:
                        for n in ptiles:
                            emit_proj_tile(1, n)
                        ptiles = []
                        emit_vaug_chunks(1, 0, nkv[1])
            drain_av(len(pend_av))

    nc.compile()
    return nc


_module_cache = {}


def _get_module(lens):
    key = tuple((l + P - 1) // P for l in lens) + tuple(l % P == 0 for l in lens)
    if key not in _module_cache:
        _module_cache[key] = _build(lens)
    return _module_cache[key]


def kernel(x, lengths, Wq, Wk, Wv, Wp, bp):
    import ml_dtypes
    from concourse.bass_utils import run_bass_kernel_spmd

    F8 = (ml_dtypes.float8_e4m3fn if hasattr(ml_dtypes, 'float8_e4m3fn')
          else ml_dtypes.float8_e4m3)
    BF = ml_dtypes.bfloat16

    x = np.asarray(x, dtype=np.float32)
    lens = tuple(int(np.clip(int(v), 1, T)) for v in np.asarray(lengths).reshape(-1))
    Wq = np.asarray(Wq, dtype=np.float32)
    Wk = np.asarray(Wk, dtype=np.float32)
    Wv = np.asarray(Wv, dtype=np.float32)
    Wp = np.asarray(Wp, dtype=np.float32)
    bp = np.asarray(bp, dtype=np.float32)

    nc = _get_module(lens)

    # x8: [128, kc, 2, B*T] fp8 planes (hi, lo)
    xt = np.ascontiguousarray(x.reshape(B * T, C).T)          # [C, B*T]
    xr = xt.reshape(KC_N, P, B * T).transpose(1, 0, 2)        # [P, kc, B*T]
    x1 = xr.astype(F8)
    x2 = (xr - x1.astype(np.float32)).astype(F8)
    x8 = np.stack([x1, x2], axis=2)                           # [P, kc, 2, B*T]

    km = np.zeros((P, B), dtype=np.float32)
    for b in range(B):
        pc = (lens[b] - 1) // P
        idx = pc * P + np.arange(P)
        km[:, b] = np.where(idx < lens[b], 0.0, NEG).astype(np.float32)

    def wsplit(Wfull, h0):
        # per-core [C, 128] slice, scaled x64, hi/lo fp8 split
        Wc = np.concatenate([Wfull[h0 + i] for i in range(HP)], axis=1) * 64.0
        Wr = Wc.reshape(KC_N, P, M).transpose(1, 0, 2)        # [P, kc, M]
        W1 = Wr.astype(F8)
        W2 = (Wr - W1.astype(np.float32)).astype(F8)
        wx = np.stack([W2, W1], axis=2)                       # planes (W2, W1)
        return np.ascontiguousarray(W1), np.ascontiguousarray(wx)

    in_maps = []
    for core in range(NCORES):
        h0 = core * HP
        wqh, wqx = wsplit(Wq, h0)
        wkh, wkx = wsplit(Wk, h0)
        wvh, wvx = wsplit(Wv, h0)
        in_maps.append({
            "x8": x8.view(np.uint8),
            "wqhi": wqh.view(np.uint8), "wqx": wqx.view(np.uint8),
            "wkhi": wkh.view(np.uint8), "wkx": wkx.view(np.uint8),
            "wvhi": wvh.view(np.uint8), "wvx": wvx.view(np.uint8),
            "wp": np.ascontiguousarray(
                Wp[h0 * DH:(h0 + HP) * DH, :].astype(BF)).view(np.uint16),
            "km": km,
        })

    res = run_bass_kernel_spmd(nc, in_maps, list(range(NCORES)))

    out = np.zeros((B * T, C), dtype=np.float32)
    for r in res.results:
        out += np.asarray(r["out"]).view(np.float16).astype(np.float32) \
            if r["out"].dtype != np.float16 else r["out"].astype(np.float32)
    out = out.reshape(B, T, C)
    for b in range(B):
        out[b, lens[b]:, :] = 0.0
    out += bp
    return out


# revision 25
# speedup vs baseline: 1.1538x; 1.1538x over previous
"""Trainium2 Bass kernel for multi-head attention (B=2, T=2048, C=1024, H=16, DH=64).

Sharding: tensor-parallel over heads. Each of the 8 cores computes 2 heads:
q/k/v projections for its heads, attention, and a partial output projection
(its 128-column slice of the concat-head dim against its 128-row slice of Wp).
The host sums the 8 fp16 partial outputs in fp32 and adds the bias.

Numerics/engine strategy (validated against a float64 oracle, rel err ~9e-3
vs the 2e-2 gate):
  - projections run as fp8e4 DoubleRow matmuls on a 3-term hi/lo split:
    x = x1+x2, W*64 = W1+W2, q ~ (x1W1) + (x2W1 + x1W2), with the two cross
    terms packed into the two DoubleRow planes of a single instruction.
    6 instrs/chunk-pair-equivalent vs 8 fp32r chunks -> 0.75x PE cycles, and
    x streams from HBM as two fp8 planes (half the fp32 bytes).
  - scores run as fp8e4 DoubleRow with q split exact (q1,q2 planes) and k
    single-quantized: s = k1^T(q1+q2) at 0.5 cycles/row (2x fp32r). Both
    heads live on the free dim of [64,2,2,span] tiles so every matmul uses
    tile position (0,0) (alternating positions with DoubleRow faults).
  - exp on ACT emits bf16 tiles, two key chunks per instruction where the
    key mask allows, scale = 0.125/64 folded with the fp8 evac scales.
  - attention@V is transposed: exp tile is the stationary operand (128-query
    slices), [v|1] bf16 is moving (65-wide) -> full 128-partition output
    utilization plus a free softmax denominator column; normalization is a
    per-partition reciprocal+mul on DVE (no gpsimd broadcast).
  - normalized heads are re-transposed (bf16, via identity) and multiplied
    against bf16 Wp; psum is evacuated to fp16 on the otherwise-idle gpsimd
    engine and DMA'd out as fp16.
"""

from contextlib import ExitStack

import numpy as np

B, T, C, H, DH = 2, 2048, 1024, 16, 64
NCORES = 8
HP = H // NCORES          # heads per core
M = HP * DH               # 128 = packed head dim per core
P = 128                   # partitions / contraction chunk
QT = 512                  # query/token tile (free dim)
NEG = -30000.0            # additive mask value (exp(NEG + anything small) == 0)
KC_N = C // P             # contraction chunks for projections
ESC = 0.125 / 64.0        # exp scale: 1/sqrt(dh) / (q*8 * k*8)


def _build(lens, t=T, c=C):
    """Build the per-core Bass module for batch lengths `lens` (tuple of B ints)."""
    import concourse.mybir as mybir
    import concourse.tile as tile
    from concourse import bacc
    from concourse.masks import make_identity

    f32 = mybir.dt.float32
    bf16 = mybir.dt.bfloat16
    f16 = mybir.dt.float16
    fp8 = mybir.dt.float8e4
    AF = mybir.ActivationFunctionType
    PM = mybir.MatmulPerfMode
    ALU = mybir.AluOpType

    qt = min(QT, t)
    nkv = [(l + P - 1) // P for l in lens]         # valid key chunks == token blocks
    partial = [l % P != 0 for l in lens]
    crop = [n * P for n in nkv]                    # token coverage per batch
    nq = [(cr + qt - 1) // qt for cr in crop]      # query tiles per batch
    nkv_max = max(nkv)

    def tiw(b, i):
        return min(crop[b] - i * qt, qt)           # multiples of 128

    nc = bacc.Bacc("TRN2", target_bir_lowering=False, debug=False,
                   num_devices=NCORES)

    x8_d = nc.dram_tensor("x8", [P, KC_N, 2, B * t], fp8, kind="ExternalInput").ap()
    w_hi_d = [nc.dram_tensor(f"w{n}hi", [P, KC_N, M], fp8, kind="ExternalInput").ap()
              for n in ("q", "k", "v")]
    w_x_d = [nc.dram_tensor(f"w{n}x", [P, KC_N, 2, M], fp8, kind="ExternalInput").ap()
             for n in ("q", "k", "v")]
    wp_d = nc.dram_tensor("wp", [M, c], bf16, kind="ExternalInput").ap()
    km_d = nc.dram_tensor("km", [P, B], f32, kind="ExternalInput").ap()
    out_d = nc.dram_tensor("out", [B * t, c], f16, kind="ExternalOutput").ap()

    with tile.TileContext(nc) as tc, ExitStack() as ctx:
        const = ctx.enter_context(tc.tile_pool(name="const", bufs=1))
        persist = ctx.enter_context(tc.tile_pool(name="persist", bufs=1))

        identb = const.tile([P, P], bf16)
        make_identity(nc, identb[:])
        kmask = const.tile([P, B], f32)
        wp_sb = const.tile([P, c], bf16)

        # q8/k8: [64 dims, plane, head, token] so both heads' score matmuls
        # sit at tile position (0,0); k planes both hold k1 (duplicated).
        q8 = persist.tile([DH, 2, HP, B * t], fp8, tag="q8")
        k8 = persist.tile([DH, 2, HP, B * t], fp8, tag="k8")
        vTb = persist.tile([P, B * t], bf16, tag="vTb")
        vaug = persist.tile([P, B, nkv_max, 2 * (DH + 1)], bf16, tag="vaug")

        with tc.tile_pool(name="wpool", bufs=1) as wpool, \
             tc.tile_pool(name="xpool", bufs=7) as xpool, \
             tc.tile_pool(name="exps", bufs=28) as expp, \
             tc.tile_pool(name="aob", bufs=4) as aobp, \
             tc.tile_pool(name="aot", bufs=4) as aotp, \
             tc.tile_pool(name="stage", bufs=3) as stage, \
             tc.tile_pool(name="recp", bufs=8) as recp, \
             tc.tile_pool(name="work", bufs=2, space="PSUM") as workp, \
             tc.tile_pool(name="psc", bufs=2, space="PSUM") as pscp, \
             tc.tile_pool(name="pav", bufs=2, space="PSUM") as pavp:

            w_hi = []
            w_x = []
            for i, n in enumerate(("q", "k", "v")):
                whi = wpool.tile([P, KC_N, M], fp8, tag=f"w{n}hi", name=f"w{n}hi")
                wx = wpool.tile([P, KC_N, 2, M], fp8, tag=f"w{n}x", name=f"w{n}x")
                w_hi.append(whi)
                w_x.append(wx)
            nc.vector.memset(vaug[:], 1.0)


            # Warm-up: dependency-free matmuls release the PE clock gate,
            # a dummy Exp preloads the ACT table set
            warm = workp.tile([P, qt], f32, tag="work", name="warm")
            for i in range(17):
                nc.tensor.matmul(warm[:, 0:P], identb[:], identb[:],
                                 start=(i == 0), stop=(i == 16))
            dummy = const.tile([P, P], f32, name="dummy")
            nc.scalar.activation(dummy[:], identb[:], AF.Exp)

            def emit_proj_dma(b, n):
                tok0 = b * t + n * qt
                tw = tiw(b, n)
                xtile = xpool.tile([P, KC_N, 2, qt], fp8, tag="x", name="xtile")
                if b == 0 and n == 0:
                    # weight DMAs first, x tile split so matmuls start early
                    nc.sync.dma_start(w_hi[0][:], w_hi_d[0][:])
                    nc.sync.dma_start(w_x[0][:], w_x_d[0][:])
                    nc.sync.dma_start(
                        xtile[:, 0:2, :, 0:tw], x8_d[:, 0:2, :, tok0:tok0 + tw])
                    nc.sync.dma_start(w_hi[1][:], w_hi_d[1][:])
                    nc.sync.dma_start(w_x[1][:], w_x_d[1][:])
                    nc.sync.dma_start(w_hi[2][:], w_hi_d[2][:])
                    nc.sync.dma_start(w_x[2][:], w_x_d[2][:])
                    nc.sync.dma_start(
                        xtile[:, 2:KC_N, :, 0:tw], x8_d[:, 2:KC_N, :, tok0:tok0 + tw])
                    nc.sync.dma_start(kmask[:], km_d[:])
                    nc.sync.dma_start(wp_sb[:], wp_d[:])
                else:
                    nc.sync.dma_start(
                        xtile[:, :, :, 0:tw], x8_d[:, :, :, tok0:tok0 + tw])
                return xtile

            def emit_proj_mm(b, n, i, xtile):
                tok0 = b * t + n * qt
                tw = tiw(b, n)
                ps = workp.tile([P, qt], f32, tag="work", name="ps")
                # main term: x1@W1, two chunks per DoubleRow instr
                for k in range(KC_N // 2):
                    nc.tensor.matmul(
                        ps[:, 0:tw],
                        w_hi[i][:, 2 * k:2 * k + 2, :],
                        xtile[:, 2 * k:2 * k + 2, 0, 0:tw],
                        start=(k == 0), stop=False, perf_mode=PM.DoubleRow)
                # cross terms: planes (W2,x1),(W1,x2) per chunk
                for k in range(KC_N):
                    nc.tensor.matmul(
                        ps[:, 0:tw],
                        w_x[i][:, k, :, :],
                        xtile[:, k, :, 0:tw],
                        start=False, stop=(k == KC_N - 1),
                        perf_mode=PM.DoubleRow)
                span = slice(tok0, tok0 + tw)
                use_act = b == 0   # ACT helps only while its exp stream is light
                if i == 0:      # q: plane0 = fp8(ps/8), resid on DVE
                    h0, h1 = slice(0, DH), slice(DH, 2 * DH)
                    nc.vector.tensor_scalar(
                        q8[:, 0, 0, span], ps[h0, 0:tw], 0.125, None, ALU.mult)
                    if use_act:
                        nc.scalar.activation(q8[:, 0, 1, span], ps[h1, 0:tw],
                                             AF.Copy, scale=0.125)
                    else:
                        nc.vector.tensor_scalar(
                            q8[:, 0, 1, span], ps[h1, 0:tw], 0.125, None,
                            ALU.mult)
                    for h in range(HP):
                        hsl = slice(h * DH, (h + 1) * DH)
                        nc.vector.scalar_tensor_tensor(
                            q8[:, 1, h, span], ps[hsl, 0:tw], 0.125,
                            q8[:, 0, h, span], ALU.mult, ALU.subtract)
                elif i == 1:    # k: fp8 into plane0, gpsimd dups to plane1
                    h0, h1 = slice(0, DH), slice(DH, 2 * DH)
                    if use_act:
                        nc.scalar.activation(k8[:, 0, 0, span], ps[h0, 0:tw],
                                             AF.Copy, scale=0.125)
                    else:
                        nc.vector.tensor_scalar(
                            k8[:, 0, 0, span], ps[h0, 0:tw], 0.125, None,
                            ALU.mult)
                    nc.vector.tensor_scalar(
                        k8[:, 0, 1, span], ps[h1, 0:tw], 0.125, None, ALU.mult)
                    nc.gpsimd.tensor_copy(k8[:, 1, :, span], k8[:, 0, :, span])
                else:           # v: bf16 at true scale
                    nc.vector.tensor_scalar(
                        vTb[:, span], ps[:, 0:tw], 1.0 / 64.0, None,
                        ALU.mult)

            def emit_proj_tile(b, n):
                xtile = emit_proj_dma(b, n)
                for i in range(3):
                    emit_proj_mm(b, n, i, xtile)

            def emit_vaug_chunks(b, k0, k1):
                for k in range(k0, k1):
                    key0 = b * t + k * P
                    pt = workp.tile([P, qt], bf16, tag="work", name="pt")
                    nc.tensor.transpose(pt[:, 0:P], vTb[:, key0:key0 + P],
                                        identb[:])
                    dst = vaug[:, b, k, :].rearrange(
                        "p (g w) -> p g w", g=2)[:, :, 0:DH]
                    nc.vector.tensor_copy(
                        dst, pt[:, 0:P].rearrange("p (g w) -> p g w", g=2))

            def _exp_single(b, qw, sck, k, etiles, bias):
                et = expp.tile([P, HP, qt], bf16, tag="et", name="et")
                src = sck[:, :, 0:qw]
                dst = et[:, :, 0:qw]
                if bias is None:
                    nc.scalar.activation(dst, src, AF.Exp, scale=ESC)
                else:
                    nc.scalar.activation(dst, src, AF.Exp, bias=bias,
                                         scale=ESC)
                etiles.append((k, et))

            def emit_scores_chunks(b, q, k0, k1, etiles, pend):
                q0 = b * t + q * qt
                qw = tiw(b, q)
                for k in range(k0, k1):
                    key0 = b * t + k * P
                    sck = pscp.tile([P, HP, qt], f32, tag="sc", name="sck")
                    for h in range(HP):
                        nc.tensor.matmul(
                            sck[:, h, 0:qw],
                            k8[:, :, h, key0:key0 + P],
                            q8[:, :, h, q0:q0 + qw],
                            start=True, stop=True, perf_mode=PM.DoubleRow)
                    masked = partial[b] and k == nkv[b] - 1
                    _exp_single(b, qw, sck, k, etiles,
                                kmask[:, b:b + 1] if masked else None)

            def emit_avh(b, q, qb, etiles):
                # AV for both heads of one 128-query block + normalize.
                # Both heads share one psum bank: h0 at cols 0:65, h1 at
                # 256:321; one accumulation group (start on first, stop on
                # last) -- pending-zero bytes are zeroed on first touch.
                col0 = qb * P
                aob = aobp.tile([P, P], bf16, tag="aob", name="aob")
                pa = pavp.tile([P, qt], f32, tag="pa", name="pa")
                nkb = nkv[b]
                for h in range(HP):
                    c0 = h * 2 * P
                    for idx, (k, et) in enumerate(etiles):
                        nc.tensor.matmul(
                            pa[:, c0:c0 + DH + 1],
                            et[:, h, col0:col0 + P],
                            vaug[:, b, k, h * (DH + 1):(h + 1) * (DH + 1)],
                            start=(h == 0 and idx == 0),
                            stop=(h == HP - 1 and idx == nkb - 1))
                rec = recp.tile([P, 2], f32, tag="rec", name="rec")
                den = pa[:].rearrange("p (g w) -> p g w", g=2)[:, :, DH:DH + 1]
                nc.vector.reciprocal(rec[:], den)
                for h in range(HP):
                    c0 = h * 2 * P
                    nc.vector.tensor_scalar(
                        aob[:, h * DH:(h + 1) * DH], pa[:, c0:c0 + DH],
                        rec[:, h:h + 1], None, ALU.mult)
                return aob

            def emit_top(b, q, qb, aob, final):
                # transpose + output projection + fp16 evac/DMA for one block
                pat = workp.tile([P, qt], bf16, tag="work", name="pat")
                nc.tensor.transpose(pat[:, 0:P], aob[:], identb[:])
                aotT = aotp.tile([P, P], bf16, tag="aotT", name="aotT")
                nc.vector.tensor_copy(aotT[:], pat[:, 0:P])
                blk = q * (qt // P) + qb
                tok0 = b * t + blk * P
                st = stage.tile([P, c], f16, tag="st", name="st")
                qn = c // 2
                for nn in range(2):
                    po = workp.tile([P, qt], f32, tag="work", name="po")
                    nc.tensor.matmul(
                        po[:, 0:qn], aotT[:], wp_sb[:, nn * qn:(nn + 1) * qn],
                        start=True, stop=True)
                    nc.vector.tensor_copy(st[:, nn * qn:(nn + 1) * qn],
                                          po[:, 0:qn])
                if final:
                    for j in range(4):
                        cs = slice(j * (c // 4), (j + 1) * (c // 4))
                        nc.sync.dma_start(out_d[tok0:tok0 + P, cs], st[:, cs])
                else:
                    nc.sync.dma_start(out_d[tok0:tok0 + P, :], st[:])

            # ---- software-pipelined emission ----
            # Backlog of deferred PE work units (cost_ns, emit_fn); drained
            # in FIFO order between score chunk-pairs so the PE always has
            # work while ACT runs the exp stream.
            backlog = []   # (cost_ns, fn, kind); kind 'pre' = needed pre-b1

            def drain(budget_ns):
                while backlog and budget_ns > 0:
                    cost, fn, _ = backlog.pop(0)
                    fn()
                    budget_ns -= cost

            def drain_all():
                drain(float("inf"))

            def drain_prereq():
                rest = []
                for cost, fn, kind in backlog:
                    if kind == "pre":
                        fn()
                    else:
                        rest.append((cost, fn, kind))
                backlog[:] = rest

            def push_block_units(b, q, qb, etiles, final):
                holder = {}

                def do_avh(b=b, q=q, qb=qb, etiles=etiles):
                    holder["aob"] = emit_avh(b, q, qb, etiles)

                def do_top(b=b, q=q, qb=qb, final=final):
                    emit_top(b, q, qb, holder["aob"], final)

                av_ns = int(2 * nkv[b] * (DH + 1) * 0.42) + 100
                backlog.append((av_ns, do_avh, "av"))
                backlog.append((520, do_top, "av"))

            def push_proj_units(b, n):
                # DMA issued immediately (prefetch); matmuls deferred
                xtile = emit_proj_dma(b, n)
                for i in range(3):
                    def do_mm(b=b, n=n, i=i, xtile=xtile):
                        emit_proj_mm(b, n, i, xtile)
                    backlog.append((6 * tiw(b, n) * 5 // 12, do_mm, "pre"))

            def push_vaug_units(b):
                for k0 in range(0, nkv[b], 4):
                    k1 = min(k0 + 4, nkv[b])
                    def do_v(b=b, k0=k0, k1=k1):
                        emit_vaug_chunks(b, k0, k1)
                    backlog.append(((k1 - k0) * 60, do_v, "pre"))

            # b0 projections first, interleaved with early b0 score chunks
            etiles0 = []
            k_sc = 0
            for n in range(nq[0]):
                emit_proj_tile(0, n)
                hi = min(((n + 1) * qt) // P, nkv[0])
                take = min(k_sc + 2, hi)
                emit_scores_chunks(0, 0, k_sc, take, etiles0, None)
                k_sc = take
            # queue b1 projections + both batches' vaug builds behind them
            for n in range(nq[1]) if B > 1 else []:
                push_proj_units(1, n)
            push_vaug_units(0)
            if B > 1:
                push_vaug_units(1)
            while k_sc < nkv[0]:
                take = min(k_sc + 2, nkv[0])
                emit_scores_chunks(0, 0, k_sc, take, etiles0, None)
                k_sc = take
                drain(1650)

            all_tiles = [(b, q) for b in range(B) for q in range(nq[b])]
            for ti, (b, q) in enumerate(all_tiles):
                last_tile = ti == len(all_tiles) - 1
                if b == 0 and q == 0:
                    ets = etiles0
                else:
                    # scores + exp, draining backlog between chunk-pairs
                    ets = []
                    for k in range(nkv[b]):
                        emit_scores_chunks(b, q, k, k + 1, ets, None)
                        drain(700)
                nqb = tiw(b, q) // P
                for qb in range(nqb):
                    push_block_units(b, q, qb, ets,
                                     final=last_tile and qb == nqb - 1)
                if b == 0 and q == nq[0] - 1:
                    # b1 proj/vaug units must be emitted before b1 scores
                    # reference q8/k8/vaug; AV fillers stay queued
                    drain_prereq()
            drain_all()

    nc.compile()
    return nc


_module_cache = {}


def _get_module(lens):
    key = tuple((l + P - 1) // P for l in lens) + tuple(l % P == 0 for l in lens)
    if key not in _module_cache:
        _module_cache[key] = _build(lens)
    return _module_cache[key]


def kernel(x, lengths, Wq, Wk, Wv, Wp, bp):
    import ml_dtypes
    from concourse.bass_utils import run_bass_kernel_spmd

    F8 = (ml_dtypes.float8_e4m3fn if hasattr(ml_dtypes, 'float8_e4m3fn')
          else ml_dtypes.float8_e4m3)
    BF = ml_dtypes.bfloat16

    x = np.asarray(x, dtype=np.float32)
    lens = tuple(int(np.clip(int(v), 1, T)) for v in np.asarray(lengths).reshape(-1))
    Wq = np.asarray(Wq, dtype=np.float32)
    Wk = np.asarray(Wk, dtype=np.float32)
    Wv = np.asarray(Wv, dtype=np.float32)
    Wp = np.asarray(Wp, dtype=np.float32)
    bp = np.asarray(bp, dtype=np.float32)

    nc = _get_module(lens)

    # x8: [128, kc, 2, B*T] fp8 planes (hi, lo)
    xt = np.ascontiguousarray(x.reshape(B * T, C).T)          # [C, B*T]
    xr = xt.reshape(KC_N, P, B * T).transpose(1, 0, 2)        # [P, kc, B*T]
    x1 = xr.astype(F8)
    x2 = (xr - x1.astype(np.float32)).astype(F8)
    x8 = np.stack([x1, x2], axis=2)                           # [P, kc, 2, B*T]

    km = np.zeros((P, B), dtype=np.float32)
    for b in range(B):
        pc = (lens[b] - 1) // P
        idx = pc * P + np.arange(P)
        km[:, b] = np.where(idx < lens[b], 0.0, NEG).astype(np.float32)

    def wsplit(Wfull, h0):
        # per-core [C, 128] slice, scaled x64, hi/lo fp8 split
        Wc = np.concatenate([Wfull[h0 + i] for i in range(HP)], axis=1) * 64.0
        Wr = Wc.reshape(KC_N, P, M).transpose(1, 0, 2)        # [P, kc, M]
        W1 = Wr.astype(F8)
        W2 = (Wr - W1.astype(np.float32)).astype(F8)
        wx = np.stack([W2, W1], axis=2)                       # planes (W2, W1)
        return np.ascontiguousarray(W1), np.ascontiguousarray(wx)

    in_maps = []
    for core in range(NCORES):
        h0 = core * HP
        wqh, wqx = wsplit(Wq, h0)
        wkh, wkx = wsplit(Wk, h0)
        wvh, wvx = wsplit(Wv, h0)
        in_maps.append({
            "x8": x8.view(np.uint8),
            "wqhi": wqh.view(np.uint8), "wqx": wqx.view(np.uint8),
            "wkhi": wkh.view(np.uint8), "wkx": wkx.view(np.uint8),
            "wvhi": wvh.view(np.uint8), "wvx": wvx.view(np.uint8),
            "wp": np.ascontiguousarray(
                Wp[h0 * DH:(h0 + HP) * DH, :].astype(BF)).view(np.uint16),
            "km": km,
        })

    res = run_bass_kernel_spmd(nc, in_maps, list(range(NCORES)))

    out = np.zeros((B * T, C), dtype=np.float32)
    for r in res.results:
        out += np.asarray(r["out"]).view(np.float16).astype(np.float32) \
            if r["out"].dtype != np.float16 else r["out"].astype(np.float32)
    out = out.reshape(B, T, C)
    for b in range(B):
        out[b, lens[b]:, :] = 0.0
    out += bp
    return out


# revision 61
# speedup vs baseline: 1.2936x; 1.1211x over previous
"""Trainium2 Bass kernel for multi-head attention (B=2, T=2048, C=1024, H=16, DH=64).

Sharding: tensor-parallel over heads. Each of the 8 cores computes 2 heads:
q/k/v projections for its heads, attention, and a partial output projection
(its 128-column slice of the concat-head dim against its 128-row slice of Wp).
The host sums the 8 fp16 partial outputs in fp32 and adds the bias.

Numerics/engine strategy (validated against a float64 oracle, rel err ~9e-3
vs the 2e-2 gate):
  - projections run as fp8e4 DoubleRow matmuls on a 3-term hi/lo split:
    x = x1+x2, W*64 = W1+W2, q ~ (x1W1) + (x2W1 + x1W2), with the two cross
    terms packed into the two DoubleRow planes of a single instruction.
    6 instrs/chunk-pair-equivalent vs 8 fp32r chunks -> 0.75x PE cycles, and
    x streams from HBM as two fp8 planes (half the fp32 bytes).
  - scores run as fp8e4 DoubleRow with q split exact (q1,q2 planes) and k
    single-quantized: s = k1^T(q1+q2) at 0.5 cycles/row (2x fp32r). Both
    heads live on the free dim of [64,2,2,span] tiles so every matmul uses
    tile position (0,0) (alternating positions with DoubleRow faults).
  - exp on ACT emits bf16 tiles, two key chunks per instruction where the
    key mask allows, scale = 0.125/64 folded with the fp8 evac scales.
  - attention@V is transposed: exp tile is the stationary operand (128-query
    slices), [v|1] bf16 is moving (65-wide) -> full 128-partition output
    utilization plus a free softmax denominator column; normalization is a
    per-partition reciprocal+mul on DVE (no gpsimd broadcast).
  - normalized heads are re-transposed (bf16, via identity) and multiplied
    against bf16 Wp; psum is evacuated to fp16 on the otherwise-idle gpsimd
    engine and DMA'd out as fp16.
"""

from contextlib import ExitStack

import numpy as np

B, T, C, H, DH = 2, 2048, 1024, 16, 64
NCORES = 8
HP = H // NCORES          # heads per core
M = HP * DH               # 128 = packed head dim per core
P = 128                   # partitions / contraction chunk
QT = 512                  # query/token tile (free dim)
NEG = -30000.0            # additive mask value (exp(NEG + anything small) == 0)
KC_N = C // P             # contraction chunks for projections
ESC = 0.125 / 64.0        # exp scale: 1/sqrt(dh) / (q*8 * k*8)


def _build(lens, t=T, c=C):
    """Build the per-core Bass module for batch lengths `lens` (tuple of B ints)."""
    import concourse.mybir as mybir
    import concourse.tile as tile
    from concourse import bacc
    from concourse.masks import make_identity

    f32 = mybir.dt.float32
    bf16 = mybir.dt.bfloat16
    f16 = mybir.dt.float16
    fp8 = mybir.dt.float8e4
    AF = mybir.ActivationFunctionType
    PM = mybir.MatmulPerfMode
    ALU = mybir.AluOpType

    qt = min(QT, t)
    nkv = [(l + P - 1) // P for l in lens]         # valid key chunks == token blocks
    partial = [l % P != 0 for l in lens]
    crop = [n * P for n in nkv]                    # token coverage per batch
    nq = [(cr + qt - 1) // qt for cr in crop]      # query tiles per batch
    nkv_max = max(nkv)

    def tiw(b, i):
        return min(crop[b] - i * qt, qt)           # multiples of 128

    nc = bacc.Bacc("TRN2", target_bir_lowering=False, debug=False,
                   num_devices=NCORES)

    x8_d = nc.dram_tensor("x8", [P, KC_N, 2, B * t], fp8, kind="ExternalInput").ap()
    w_x_d = [nc.dram_tensor(f"w{n}x", [P, KC_N, 2, M], fp8, kind="ExternalInput").ap()
             for n in ("q", "k", "v")]
    wp_d = nc.dram_tensor("wp", [M, c], bf16, kind="ExternalInput").ap()
    km_d = nc.dram_tensor("km", [P, B], f32, kind="ExternalInput").ap()
    out_d = nc.dram_tensor("out", [B * t, c], f16, kind="ExternalOutput").ap()

    with tile.TileContext(nc) as tc, ExitStack() as ctx:
        const = ctx.enter_context(tc.tile_pool(name="const", bufs=1))
        persist = ctx.enter_context(tc.tile_pool(name="persist", bufs=1))

        identb = const.tile([P, P], bf16)
        make_identity(nc, identb[:])
        kmask = const.tile([P, B], f32)
        wp_sb = const.tile([P, c], bf16)

        # q8/k8: [64 dims, plane, head, token] so both heads' score matmuls
        # sit at tile position (0,0); k planes both hold k1 (duplicated).
        q8 = persist.tile([DH, 2, HP, B * t], fp8, tag="q8")
        k8 = persist.tile([DH, 2, HP, B * t], fp8, tag="k8")
        vTb = persist.tile([P, B * t], bf16, tag="vTb")
        vaug = persist.tile([P, B, nkv_max, 2 * (DH + 1)], bf16, tag="vaug")

        with tc.tile_pool(name="wpool", bufs=1) as wpool, \
             tc.tile_pool(name="xpool", bufs=7) as xpool, \
             tc.tile_pool(name="exps", bufs=34) as expp, \
             tc.tile_pool(name="aob", bufs=9) as aobp, \
             tc.tile_pool(name="aot", bufs=9) as aotp, \
             tc.tile_pool(name="stage", bufs=6) as stage, \
             tc.tile_pool(name="recp", bufs=16) as recp, \
             tc.tile_pool(name="work", bufs=2, space="PSUM") as workp, \
             tc.tile_pool(name="psc", bufs=2, space="PSUM") as pscp, \
             tc.tile_pool(name="pav", bufs=2, space="PSUM") as pavp:

            w_x = []
            for i, n in enumerate(("q", "k", "v")):
                wx = wpool.tile([P, KC_N, 2, M], fp8, tag=f"w{n}x", name=f"w{n}x")
                w_x.append(wx)
            nc.vector.memset(vaug[:], 1.0)


            # Warm-up: dependency-free matmuls release the PE clock gate,
            # a dummy Exp preloads the ACT table set
            warm = workp.tile([P, qt], f32, tag="work", name="warm")
            for i in range(17):
                nc.tensor.matmul(warm[:, 0:P], identb[:], identb[:],
                                 start=(i == 0), stop=(i == 16))
            dummy = const.tile([P, P], f32, name="dummy")
            nc.scalar.activation(dummy[:], identb[:], AF.Exp)

            def emit_proj_dma(b, n):
                tok0 = b * t + n * qt
                tw = tiw(b, n)
                xtile = xpool.tile([P, KC_N, 2, qt], fp8, tag="x", name="xtile")
                if b == 0 and n == 0:
                    # cross weights + half the x tile first: the cross-half-A
                    # matmuls (chunks 0:4) start as soon as these land
                    half = KC_N // 2
                    nc.sync.dma_start(w_x[0][:], w_x_d[0][:])
                    nc.sync.dma_start(
                        xtile[:, 0:half, :, 0:tw],
                        x8_d[:, 0:half, :, tok0:tok0 + tw])
                    nc.sync.dma_start(w_x[1][:], w_x_d[1][:])
                    nc.sync.dma_start(
                        xtile[:, half:KC_N, :, 0:tw],
                        x8_d[:, half:KC_N, :, tok0:tok0 + tw])
                    nc.sync.dma_start(w_x[2][:], w_x_d[2][:])
                    nc.sync.dma_start(kmask[:], km_d[:])
                    nc.sync.dma_start(wp_sb[:], wp_d[:])
                elif b == 0:
                    # still on the startup critical path: halves let the
                    # cross matmuls start before the full tile lands
                    half = KC_N // 2
                    nc.sync.dma_start(
                        xtile[:, 0:half, :, 0:tw],
                        x8_d[:, 0:half, :, tok0:tok0 + tw])
                    nc.sync.dma_start(
                        xtile[:, half:KC_N, :, 0:tw],
                        x8_d[:, half:KC_N, :, tok0:tok0 + tw])
                else:
                    # prefetched far ahead: one descriptor, less HWDGE serial
                    nc.sync.dma_start(
                        xtile[:, :, :, 0:tw], x8_d[:, :, :, tok0:tok0 + tw])
                return xtile

            def emit_proj_mm_part(b, n, i, xtile, holder, part):
                # emitted in order 0,1,2 = cross half A, cross half B,
                # main term (x1@W1) + evac; cross half A needs only the
                # first half of the x tile, so the pipeline starts earlier
                tw = tiw(b, n)
                if part < 2:
                    if part == 0:
                        ps = workp.tile([P, qt], f32, tag="work", name="ps")
                        holder[i] = ps
                    else:
                        ps = holder[i]
                    k0 = part * (KC_N // 2)
                    for k in range(k0, k0 + KC_N // 2):
                        nc.tensor.matmul(
                            ps[:, 0:tw],
                            w_x[i][:, k, :, :],
                            xtile[:, k, :, 0:tw],
                            start=(k == 0), stop=False,
                            perf_mode=PM.DoubleRow)
                    return
                ps = holder[i]
                for k in range(KC_N // 2):
                    nc.tensor.matmul(
                        ps[:, 0:tw],
                        w_x[i][:, 2 * k:2 * k + 2, 1, :],
                        xtile[:, 2 * k:2 * k + 2, 0, 0:tw],
                        start=False, stop=(k == KC_N // 2 - 1),
                        perf_mode=PM.DoubleRow)
                emit_proj_evac(b, n, i, ps)

            def emit_proj_evac(b, n, i, ps):
                tok0 = b * t + n * qt
                tw = tiw(b, n)
                span = slice(tok0, tok0 + tw)
                use_act = b == 0   # ACT helps only while its exp stream is light
                if i == 0:      # q: plane0 = fp8(ps/8), resid on DVE
                    h0, h1 = slice(0, DH), slice(DH, 2 * DH)
                    nc.vector.tensor_scalar(
                        q8[:, 0, 0, span], ps[h0, 0:tw], 0.125, None, ALU.mult)
                    if use_act:
                        nc.scalar.activation(q8[:, 0, 1, span], ps[h1, 0:tw],
                                             AF.Copy, scale=0.125)
                    else:
                        nc.vector.tensor_scalar(
                            q8[:, 0, 1, span], ps[h1, 0:tw], 0.125, None,
                            ALU.mult)
                    for h in range(HP):
                        hsl = slice(h * DH, (h + 1) * DH)
                        nc.vector.scalar_tensor_tensor(
                            q8[:, 1, h, span], ps[hsl, 0:tw], 0.125,
                            q8[:, 0, h, span], ALU.mult, ALU.subtract)
                elif i == 1:    # k: fp8 into plane0, gpsimd dups to plane1
                    h0, h1 = slice(0, DH), slice(DH, 2 * DH)
                    if use_act:
                        nc.scalar.activation(k8[:, 0, 0, span], ps[h0, 0:tw],
                                             AF.Copy, scale=0.125)
                    else:
                        nc.vector.tensor_scalar(
                            k8[:, 0, 0, span], ps[h0, 0:tw], 0.125, None,
                            ALU.mult)
                    nc.vector.tensor_scalar(
                        k8[:, 0, 1, span], ps[h1, 0:tw], 0.125, None, ALU.mult)
                    nc.gpsimd.tensor_copy(k8[:, 1, :, span], k8[:, 0, :, span])
                else:           # v: bf16 at true scale
                    if b == 1:
                        # transition window: ACT has slack, DVE is the choke
                        nc.scalar.activation(vTb[:, span], ps[:, 0:tw],
                                             AF.Copy, scale=1.0 / 64.0)
                    else:
                        nc.vector.tensor_scalar(
                            vTb[:, span], ps[:, 0:tw], 1.0 / 64.0, None,
                            ALU.mult)

            def emit_proj_mm(b, n, i, xtile):
                holder = {}
                for part in range(3):
                    emit_proj_mm_part(b, n, i, xtile, holder, part)

            def emit_proj_tile(b, n):
                xtile = emit_proj_dma(b, n)
                for i in range(3):
                    emit_proj_mm(b, n, i, xtile)

            def emit_vaug_chunks(b, k0, k1):
                for k in range(k0, k1):
                    key0 = b * t + k * P
                    pt = workp.tile([P, qt], bf16, tag="work", name="pt")
                    nc.tensor.transpose(pt[:, 0:P], vTb[:, key0:key0 + P],
                                        identb[:])
                    dst = vaug[:, b, k, :].rearrange(
                        "p (g w) -> p g w", g=2)[:, :, 0:DH]
                    nc.vector.tensor_copy(
                        dst, pt[:, 0:P].rearrange("p (g w) -> p g w", g=2))

            def _exp_single(b, qw, sck, k, etiles, bias):
                et = expp.tile([P, HP, qt], bf16, tag="et", name="et")
                src = sck[:, :, 0:qw]
                dst = et[:, :, 0:qw]
                if bias is None:
                    nc.scalar.activation(dst, src, AF.Exp, scale=ESC)
                else:
                    nc.scalar.activation(dst, src, AF.Exp, bias=bias,
                                         scale=ESC)
                etiles.append((k, et))

            def emit_scores_chunks(b, q, k0, k1, etiles, pend):
                q0 = b * t + q * qt
                qw = tiw(b, q)
                # crop to valid queries at 64-granularity: columns past the
                # batch length feed rows the host discards
                ew = min(qw, max(64, (lens[b] - q * qt + 63) // 64 * 64))
                for k in range(k0, k1):
                    key0 = b * t + k * P
                    sck = pscp.tile([P, HP, qt], f32, tag="sc", name="sck")
                    for h in range(HP):
                        nc.tensor.matmul(
                            sck[:, h, 0:ew],
                            k8[:, :, h, key0:key0 + P],
                            q8[:, :, h, q0:q0 + ew],
                            start=True, stop=True, perf_mode=PM.DoubleRow)
                    masked = partial[b] and k == nkv[b] - 1
                    _exp_single(b, ew, sck, k, etiles,
                                kmask[:, b:b + 1] if masked else None)

            endgame = [False]   # true once the exp stream is fully emitted

            def emit_avh(b, q, qb, etiles, final=False):
                # AV for both heads of one 128-query block + normalize.
                # Both heads share one psum bank: h0 at cols 0:65, h1 at
                # 256:321; one accumulation group (start on first, stop on
                # last) -- pending-zero bytes are zeroed on first touch.
                col0 = qb * P
                aob = aobp.tile([P, P], bf16, tag="aob", name="aob")
                pa = pavp.tile([P, qt], f32, tag="pa", name="pa")
                nkb = nkv[b]
                for h in range(HP):
                    c0 = h * 2 * P
                    for idx, (k, et) in enumerate(etiles):
                        nc.tensor.matmul(
                            pa[:, c0:c0 + DH + 1],
                            et[:, h, col0:col0 + P],
                            vaug[:, b, k, h * (DH + 1):(h + 1) * (DH + 1)],
                            start=(h == 0 and idx == 0),
                            stop=(h == HP - 1 and idx == nkb - 1))
                rec = recp.tile([P, 2], f32, tag="rec", name="rec")
                den = pa[:].rearrange("p (g w) -> p g w", g=2)[:, :, DH:DH + 1]
                nc.vector.reciprocal(rec[:], den)
                for h in range(HP):
                    c0 = h * 2 * P
                    if endgame[0]:   # ACT is idle after its last exp
                        nc.scalar.activation(
                            aob[:, h * DH:(h + 1) * DH], pa[:, c0:c0 + DH],
                            AF.Copy, scale=rec[:, h:h + 1])
                    else:
                        nc.vector.tensor_scalar(
                            aob[:, h * DH:(h + 1) * DH], pa[:, c0:c0 + DH],
                            rec[:, h:h + 1], None, ALU.mult)
                return aob

            def emit_top(b, q, qb, aob, final):
                # transpose + output projection + fp16 evac/DMA for one block
                pat = workp.tile([P, qt], bf16, tag="work", name="pat")
                nc.tensor.transpose(pat[:, 0:P], aob[:], identb[:])
                aotT = aotp.tile([P, P], bf16, tag="aotT", name="aotT")
                nc.vector.tensor_copy(aotT[:], pat[:, 0:P])
                blk = q * (qt // P) + qb
                tok0 = b * t + blk * P
                st = stage.tile([P, c], f16, tag="st", name="st")
                qn = c // 2
                for nn in range(2):
                    po = workp.tile([P, qt], f32, tag="work", name="po")
                    nc.tensor.matmul(
                        po[:, 0:qn], aotT[:], wp_sb[:, nn * qn:(nn + 1) * qn],
                        start=True, stop=True)
                    if endgame[0]:
                        nc.scalar.activation(st[:, nn * qn:(nn + 1) * qn],
                                             po[:, 0:qn], AF.Copy)
                    else:
                        nc.vector.tensor_copy(st[:, nn * qn:(nn + 1) * qn],
                                              po[:, 0:qn])
                    if final:
                        nc.sync.dma_start(
                            out_d[tok0:tok0 + P, nn * qn:(nn + 1) * qn],
                            st[:, nn * qn:(nn + 1) * qn])
                if not final:
                    nc.sync.dma_start(out_d[tok0:tok0 + P, :], st[:])

            # ---- software-pipelined emission ----
            # Backlog of deferred PE work units (cost_ns, emit_fn); drained
            # in FIFO order between score chunk-pairs so the PE always has
            # work while ACT runs the exp stream.
            backlog = []   # (cost_ns, fn, kind); kind 'pre' = needed pre-b1

            def drain(budget_ns):
                while backlog and budget_ns > 0:
                    cost, fn, _ = backlog.pop(0)
                    fn()
                    budget_ns -= cost

            def drain_all():
                drain(float("inf"))

            def drain_prereq(on_unit=None):
                rest = []
                for idx, (cost, fn, kind) in enumerate(backlog):
                    if kind.startswith("pre"):
                        fn()
                        if on_unit is not None:
                            on_unit(kind)
                    else:
                        rest.append((cost, fn, kind))
                backlog[:] = rest

            def push_block_units(b, q, qb, etiles, final):
                holder = {}

                def do_avh(b=b, q=q, qb=qb, etiles=etiles, final=final):
                    holder["aob"] = emit_avh(b, q, qb, etiles, final)

                def do_top(b=b, q=q, qb=qb, final=final):
                    emit_top(b, q, qb, holder["aob"], final)

                av_ns = int(2 * nkv[b] * (DH + 1) * 0.42) + 100
                backlog.append((av_ns, do_avh, "av"))
                backlog.append((520, do_top, "av"))

            def push_proj_units(b, n):
                # DMA issued immediately (prefetch); matmuls deferred
                xtile = emit_proj_dma(b, n)
                units = {}
                for i in range(3):
                    holder = {}
                    lst = []
                    for part in range(3):
                        kind = "pre"
                        if i == 1 and part == 2:
                            kind = f"pre:k{n}"   # k evac done marker
                        def do_mm(b=b, n=n, i=i, xtile=xtile,
                                  holder=holder, part=part):
                            emit_proj_mm_part(b, n, i, xtile, holder, part)
                        lst.append((1800 if part == 2 else
                                    2 * tiw(b, n) * 5 // 12, do_mm, kind))
                    units[i] = lst
                return units

            def push_vaug_units(b):
                for k0 in range(0, nkv[b], 4):
                    k1 = min(k0 + 4, nkv[b])
                    def do_v(b=b, k0=k0, k1=k1):
                        emit_vaug_chunks(b, k0, k1)
                    backlog.append(((k1 - k0) * 60, do_v, "pre"))

            # b0 projections first, interleaved with early b0 score chunks
            etiles0 = []
            k_sc = 0
            for n in range(nq[0]):
                emit_proj_tile(0, n)
                hi = min(((n + 1) * qt) // P, nkv[0])
                take = min(k_sc + 2, hi)
                emit_scores_chunks(0, 0, k_sc, take, etiles0, None)
                k_sc = take
            # queue b1 projections: q/k for every tile first (they gate the
            # b1 exp stream), then v parts and the vaug builds
            push_vaug_units(0)
            b1_units = [push_proj_units(1, n) for n in
                        (range(nq[1]) if B > 1 else [])]
            for u in b1_units:
                backlog.extend(u[0])
                backlog.extend(u[1])
            for u in b1_units:
                backlog.extend(u[2])
            if B > 1:
                push_vaug_units(1)
            while k_sc < nkv[0]:
                take = min(k_sc + 2, nkv[0])
                emit_scores_chunks(0, 0, k_sc, take, etiles0, None)
                k_sc = take
                drain(1650)

            all_tiles = [(b, q) for b in range(B) for q in range(nq[b])]
            for ti, (b, q) in enumerate(all_tiles):
                last_tile = ti == len(all_tiles) - 1
                if b == 0 and q == 0:
                    ets = etiles0
                else:
                    # scores + exp, draining backlog between chunk-pairs
                    ets = []
                    for k in range(nkv[b]):
                        emit_scores_chunks(b, q, k, k + 1, ets, None)
                        drain(700)
                nqb = tiw(b, q) // P
                for qb in range(nqb):
                    push_block_units(b, q, qb, ets,
                                     final=last_tile and qb == nqb - 1)
                if b == 0 and q == nq[0] - 1:
                    # b1 proj/vaug units must be emitted before b1 scores
                    # reference q8/k8/vaug; b1 tile-0 scores interleave as
                    # k-coverage lands; AV fillers stay queued
                    drain_prereq(on_prereq)
            endgame[0] = True
            if len(backlog) >= 2:
                # final block's AVH+TOP first: its DMA descriptors beat the
                # stragglers into the HWDGE queue; straggler PE work overlaps
                backlog[:] = backlog[-2:] + backlog[:-2]
            drain_all()

    nc.compile()
    return nc


_module_cache = {}


def _get_module(lens):
    key = tuple((l + P - 1) // P for l in lens) + tuple(l % P == 0 for l in lens)
    if key not in _module_cache:
        _module_cache[key] = _build(lens)
    return _module_cache[key]


def kernel(x, lengths, Wq, Wk, Wv, Wp, bp):
    import ml_dtypes
    from concourse.bass_utils import run_bass_kernel_spmd

    F8 = (ml_dtypes.float8_e4m3fn if hasattr(ml_dtypes, 'float8_e4m3fn')
          else ml_dtypes.float8_e4m3)
    BF = ml_dtypes.bfloat16

    x = np.asarray(x, dtype=np.float32)
    lens = tuple(int(np.clip(int(v), 1, T)) for v in np.asarray(lengths).reshape(-1))
    Wq = np.asarray(Wq, dtype=np.float32)
    Wk = np.asarray(Wk, dtype=np.float32)
    Wv = np.asarray(Wv, dtype=np.float32)
    Wp = np.asarray(Wp, dtype=np.float32)
    bp = np.asarray(bp, dtype=np.float32)

    nc = _get_module(lens)

    # x8: [128, kc, 2, B*T] fp8 planes (hi, lo)
    xt = np.ascontiguousarray(x.reshape(B * T, C).T)          # [C, B*T]
    xr = xt.reshape(KC_N, P, B * T).transpose(1, 0, 2)        # [P, kc, B*T]
    x1 = xr.astype(F8)
    x2 = (xr - x1.astype(np.float32)).astype(F8)
    x8 = np.stack([x1, x2], axis=2)                           # [P, kc, 2, B*T]

    km = np.zeros((P, B), dtype=np.float32)
    for b in range(B):
        pc = (lens[b] - 1) // P
        idx = pc * P + np.arange(P)
        km[:, b] = np.where(idx < lens[b], 0.0, NEG).astype(np.float32)

    def wsplit(Wfull, h0):
        # per-core [C, 128] slice, scaled x64, hi/lo fp8 split
        Wc = np.concatenate([Wfull[h0 + i] for i in range(HP)], axis=1) * 64.0
        Wr = Wc.reshape(KC_N, P, M).transpose(1, 0, 2)        # [P, kc, M]
        W1 = Wr.astype(F8)
        W2 = (Wr - W1.astype(np.float32)).astype(F8)
        wx = np.stack([W2, W1], axis=2)                       # planes (W2, W1)
        return np.ascontiguousarray(W1), np.ascontiguousarray(wx)

    in_maps = []
    for core in range(NCORES):
        h0 = core * HP
        _, wqx = wsplit(Wq, h0)
        _, wkx = wsplit(Wk, h0)
        _, wvx = wsplit(Wv, h0)
        in_maps.append({
            "x8": x8.view(np.uint8),
            "wqx": wqx.view(np.uint8),
            "wkx": wkx.view(np.uint8),
            "wvx": wvx.view(np.uint8),
            "wp": np.ascontiguousarray(
                Wp[h0 * DH:(h0 + HP) * DH, :].astype(BF)).view(np.uint16),
            "km": km,
        })

    res = run_bass_kernel_spmd(nc, in_maps, list(range(NCORES)))

    out = np.zeros((B * T, C), dtype=np.float32)
    for r in res.results:
        out += np.asarray(r["out"]).view(np.float16).astype(np.float32) \
            if r["out"].dtype != np.float16 else r["out"].astype(np.float32)
    out = out.reshape(B, T, C)
    for b in range(B):
        out[b, lens[b]:, :] = 0.0
    out += bp
    return out
